# revision 8
# baseline (speedup 1.0000x reference)
"""CfC (closed-form continuous-time) 3-layer NCP encoder on 8 Trainium2 cores.

Strategy: data-parallel over batch (256 -> 32 per core), weights replicated.
Per core the T=1024 recurrence runs fully on-chip in FEATURE-MAJOR layout:
  - state s = [h0|h1|h2] (256 units) lives as [128 part, 64] fp16 tile:
    cols 0:32 = units 0:128 ("A"), cols 32:64 = units 128:256 ("B")
  - gate matmuls keep WEIGHTS stationary (M=128 cols -> FWL), state moving:
    PSUM tile [128, 192] = blocks [ff1A|ff1B|ff2A|ff2B|tA|tB]
  - x_t + bias contributions are separate K=65 matmuls (start=True) issued
    one iteration early so they hide under ACT/DVE of the previous step
  - sigmoid folded into tanh (t-weights pre-scaled 0.5): ONE activation
    instruction per step covers all 768 pre-activations
  - blend h = ff1 + (th+1)/2*(ff2-ff1) via 3 DVE ops (2 fused
    scalar_tensor_tensor), writing the next state tile directly
  - no transposes, no state copies; cross-engine chain PE->ACT->DVE->PE
"""
import sys, os

sys.path.insert(0, "/opt/trn_rl_repo")
os.environ.setdefault("JAX_PLATFORMS", "")
os.environ.setdefault("MYCRO_LOCAL_CACHE", "1")

import numpy as np

N_CORES = 8
B, T_FULL, D_IN = 256, 1024, 64
BL = B // N_CORES  # 32
H0, H1, H2 = 135, 89, 32
NCHUNK_STEPS = 16  # x steps per DMA chunk

_NC_CACHE = {}


def _build(T):
    REP = int(os.environ.get("REPEAT", "1"))
    DBG = os.environ.get("DBG", "") == "1"
    import concourse.tile as tile
    from concourse import bacc, mybir

    F32 = mybir.dt.float32
    F16 = mybir.dt.float16
    AF = mybir.ActivationFunctionType
    OP = mybir.AluOpType

    nc = bacc.Bacc("TRN2", target_bir_lowering=False, debug=False,
                   enable_asserts=True, num_devices=N_CORES)

    NCH = T // NCHUNK_STEPS
    assert T % NCHUNK_STEPS == 0

    d_x = nc.dram_tensor("x_fm", [64, T * BL], F16, kind="ExternalInput").ap()
    d_wx = nc.dram_tensor("wx", [65, 768], F16, kind="ExternalInput").ap()
    d_wa = nc.dram_tensor("wa", [128, 768], F16, kind="ExternalInput").ap()
    d_wb = nc.dram_tensor("wb", [128, 768], F16, kind="ExternalInput").ap()
    d_wfca = nc.dram_tensor("wfca", [128, 256], F16, kind="ExternalInput").ap()
    d_wfcb = nc.dram_tensor("wfcb", [128, 768], F16, kind="ExternalInput").ap()
    d_fcb = nc.dram_tensor("fcb", [128, 2], F32, kind="ExternalInput").ap()
    d_out = nc.dram_tensor("out", [128, 64], F32, kind="ExternalOutput").ap()
    if DBG:
        d_dxr = nc.dram_tensor("dbg_xr", [65, 512], F32, kind="ExternalOutput").ap()
        d_dg = nc.dram_tensor("dbg_g", [128, 192], F32, kind="ExternalOutput").ap()
        d_dst = nc.dram_tensor("dbg_st", [128, 64], F32, kind="ExternalOutput").ap()
        d_dpg = nc.dram_tensor("dbg_pg", [128, 192], F32, kind="ExternalOutput").ap()

    CW = 32 * NCHUNK_STEPS  # x-chunk width in cols

    with tile.TileContext(nc, trace_sim=False) as tc:
        with tc.tile_pool(name="persist", bufs=1) as pp, \
             tc.tile_pool(name="psum", bufs=1, space="PSUM") as psp:
            sWX = pp.tile([65, 768], F16)
            sWA = pp.tile([128, 768], F16)
            sWB = pp.tile([128, 768], F16)
            sWfca = pp.tile([128, 256], F16)
            sWfcb = pp.tile([128, 768], F16)
            sFcb = pp.tile([128, 2], F32)
            sXR = [pp.tile([65, CW], F16, name=f"XR{i}") for i in range(4)]
            sSt = [pp.tile([128, 64], F16, name=f"St{i}") for i in range(2)]
            sG = [pp.tile([128, 192], F16, name=f"G{i}") for i in range(2)]
            sD = [pp.tile([128, 64], F16, name=f"D{i}") for i in range(2)]
            sU = [pp.tile([128, 64], F16, name=f"U{i}") for i in range(2)]
            sFin = pp.tile([128, 64], F16)
            sOut = pp.tile([128, 64], F32)

            pG = [psp.tile([128, 192], F32, name=f"PG{i}") for i in range(2)]
            pFC = psp.tile([128, 64], F32)

            # --- load weights ---
            for dst, src in [(sWX, d_wx), (sWA, d_wa), (sWB, d_wb),
                             (sWfca, d_wfca), (sWfcb, d_wfcb), (sFcb, d_fcb)]:
                nc.sync.dma_start(dst[:], src)

            # --- init ---
            for s in sSt:
                nc.vector.memset(s[:], 0.0)
            for xr in sXR:
                nc.vector.memset(xr[64:65, :], 1.0)
            for c in range(min(2, NCH)):
                nc.sync.dma_start(sXR[c][0:64, :], d_x[:, c * CW:(c + 1) * CW])

            mm = nc.tensor.matmul

            import contextlib
            rep_ctx = tc.For_i(0, REP, 1) if REP > 1 else contextlib.nullcontext()
            with rep_ctx:
              for i in range(T + 2):
                p = i % 2
                w = (i + 1) % 2

                # prefetch x chunk two ahead
                if i % NCHUNK_STEPS == 0 and i // NCHUNK_STEPS + 2 < NCH:
                    c = i // NCHUNK_STEPS + 2
                    nc.sync.dma_start(sXR[c % 4][0:64, :],
                                      d_x[:, c * CW:(c + 1) * CW])

                # --- gate matmuls: weights stationary, state moving.
                # Per PSUM region the group is [X(start), A, B(stop)] kept
                # consecutive: start=True clears has_written for the WHOLE
                # bank, so interleaving groups loses accumulation state.
                # B-half regions (odd j) only have x-weights on their first
                # 7 units (h0 tail; biases are zero), so M=7 there — the
                # A-matmul's overwrite-where-unwritten covers units 7:128.
                xi = min(i, T - 1)
                slot = (xi // NCHUNK_STEPS) % 4
                c0 = (xi % NCHUNK_STEPS) * 32
                xs = sXR[slot][0:65, c0:c0 + 32]
                for j in range(6):
                    o = pG[p][:, j * 32:(j + 1) * 32]
                    mm(o, sWX[:, j * 128:(j + 1) * 128], xs,
                       start=True, stop=False)
                    mm(o, sWA[:, j * 128:(j + 1) * 128], sSt[p][:, 0:32],
                       start=False, stop=False)
                    mm(o, sWB[:, j * 128:(j + 1) * 128], sSt[p][:, 32:64],
                       start=False, stop=True)

                # --- one activation for all gates (sigmoid via tanh) ---
                nc.scalar.activation(sG[p][:], pG[p][:], AF.Tanh)

                # --- blend: h = ff1 + (th+1)/2 * (ff2-ff1) ---
                nc.vector.tensor_sub(sD[p][:], sG[p][:, 64:128], sG[p][:, 0:64])
                nc.vector.scalar_tensor_tensor(
                    sU[p][:], sG[p][:, 128:192], 1.0, sD[p][:],
                    op0=OP.add, op1=OP.mult)
                nc.vector.scalar_tensor_tensor(
                    sSt[w][:], sU[p][:], 0.5, sG[p][:, 0:64],
                    op0=OP.mult, op1=OP.add)

                # snapshot state written by iteration T-1 (holds h0@{T-1})
                # before iteration T+1 overwrites that parity tile
                if i == T - 1:
                    nc.vector.tensor_copy(sFin[:], sSt[w][:])

                if DBG and i == 0:
                    sDbgPg = pp.tile([128, 192], F32, name="dbgpg")
                    sDbgG = pp.tile([128, 192], F32, name="dbgg")
                    sDbgSt = pp.tile([128, 64], F32, name="dbgst")
                    sDbgXr = pp.tile([65, 512], F32, name="dbgxr")
                    nc.scalar.activation(sDbgPg[:], pG[p][:], AF.Copy)
                    nc.sync.dma_start(d_dpg, sDbgPg[:])
                    nc.vector.tensor_copy(sDbgG[:], sG[p][:])
                    nc.sync.dma_start(d_dg, sDbgG[:])
                    nc.vector.tensor_copy(sDbgSt[:], sSt[w][:])
                    nc.sync.dma_start(d_dst, sDbgSt[:])
                    nc.vector.tensor_copy(sDbgXr[:], sXR[0][:, 0:512])
                    nc.sync.dma_start(d_dxr, sDbgXr[:])

            # --- final FC: out[u, b] = fc_W @ [h0|h1|h2] + fc_b ---
            # h0@{T-1} in sFin; h1@{T-1} in sSt[(T+1)%2] B rows 7:96;
            # h2@{T-1} in sSt[T%2] B rows 96:128.
            st_h1 = sSt[(T + 1) % 2]
            st_h2 = sSt[T % 2]
            for v in range(2):
                o = pFC[:, v * 32:(v + 1) * 32]
                mm(o, sWfca[:, v * 128:(v + 1) * 128], sFin[:, 0:32],
                   start=True, stop=False)
                mm(o, sWfcb[:, (0 + v) * 128:(1 + v) * 128], sFin[:, 32:64],
                   start=False, stop=False)
                mm(o, sWfcb[:, (2 + v) * 128:(3 + v) * 128], st_h1[:, 32:64],
                   start=False, stop=False)
                mm(o, sWfcb[:, (4 + v) * 128:(5 + v) * 128], st_h2[:, 32:64],
                   start=False, stop=True)
                nc.vector.tensor_scalar(sOut[:, v * 32:(v + 1) * 32], o,
                                        sFcb[:, v:v + 1], None, op0=OP.add)
            nc.sync.dma_start(d_out, sOut[:])

    nc.compile()
    return nc


def _gate_mats(inputs):
    """Dense per-gate maps M[g]: [256 state-units, 321 features]
    feature space = [x(0:64) | state(64:320) | bias(320)]."""
    f = np.float32
    M = {g: np.zeros((256, 321), f) for g in ("ff1", "ff2", "t")}
    layers = [
        (0, 64, 135, 0, 64),     # L0: input x @0, recurrent h0 @64
        (135, 135, 89, 64, 199),  # L1: input h0 @64, recurrent h1 @199
        (224, 89, 32, 199, 288),  # L2: input h1 @199, recurrent h2 @288
    ]
    for l, (r0, nin, hid, in_f0, rec_f0) in enumerate(layers):
        mask = np.asarray(inputs[f"mask{l}"], f)
        W1 = np.asarray(inputs[f"W1_{l}"], f) * mask
        W2 = np.asarray(inputs[f"W2_{l}"], f) * mask
        Wt = (np.asarray(inputs[f"Wa_{l}"], f) + np.asarray(inputs[f"Wb_{l}"], f)) * 0.5
        b1 = np.asarray(inputs[f"b1_{l}"], f)
        b2 = np.asarray(inputs[f"b2_{l}"], f)
        bt = (np.asarray(inputs[f"ba_{l}"], f) + np.asarray(inputs[f"bb_{l}"], f)) * 0.5
        for g, W, bb in (("ff1", W1, b1), ("ff2", W2, b2), ("t", Wt, bt)):
            M[g][r0:r0 + hid, in_f0:in_f0 + nin] = W[:, 0:nin]
            M[g][r0:r0 + hid, rec_f0:rec_f0 + hid] = W[:, nin:nin + hid]
            M[g][r0:r0 + hid, 320] = bb
    return M


def _prep_weights(inputs):
    f16 = np.float16
    f = np.float32
    M = _gate_mats(inputs)
    wx = np.zeros((65, 768), f)
    wa = np.zeros((128, 768), f)
    wb = np.zeros((128, 768), f)
    for gi, g in enumerate(("ff1", "ff2", "t")):
        for v in range(2):
            j = gi * 2 + v
            units = slice(v * 128, (v + 1) * 128)
            wx[0:64, j * 128:(j + 1) * 128] = M[g][units, 0:64].T
            wx[64, j * 128:(j + 1) * 128] = M[g][units, 320]
            wa[:, j * 128:(j + 1) * 128] = M[g][units, 64:192].T
            wb[:, j * 128:(j + 1) * 128] = M[g][units, 192:320].T

    fcW = np.asarray(inputs["fc_W"], f)  # [256, 256]
    fcb = np.asarray(inputs["fc_b"], f)
    wfca = np.zeros((128, 256), f)
    wfcb = np.zeros((128, 768), f)
    fcbt = np.zeros((128, 2), f)
    for v in range(2):
        rows = slice(v * 128, (v + 1) * 128)
        wfca[:, v * 128:(v + 1) * 128] = fcW[rows, 0:128].T
        full = fcW[rows, 128:256].T  # [128 K(state 128:256), 128 out]
        h0p = np.zeros((128, 128), f); h0p[0:7] = full[0:7]
        h1p = np.zeros((128, 128), f); h1p[7:96] = full[7:96]
        h2p = np.zeros((128, 128), f); h2p[96:128] = full[96:128]
        wfcb[:, (0 + v) * 128:(1 + v) * 128] = h0p
        wfcb[:, (2 + v) * 128:(3 + v) * 128] = h1p
        wfcb[:, (4 + v) * 128:(5 + v) * 128] = h2p
        fcbt[:, v] = fcb[rows]
    return {"wx": wx.astype(f16), "wa": wa.astype(f16), "wb": wb.astype(f16),
            "wfca": wfca.astype(f16), "wfcb": wfcb.astype(f16), "fcb": fcbt}


def kernel(**inputs):
    from concourse.bass_utils import run_bass_kernel_spmd

    T = inputs["x"].shape[1]
    key = (T, os.environ.get("REPEAT", "1"))
    if key not in _NC_CACHE:
        _NC_CACHE[key] = _build(T)
    nc = _NC_CACHE[key]

    shared = _prep_weights(inputs)
    x = np.asarray(inputs["x"], dtype=np.float32)
    in_maps = []
    for c in range(N_CORES):
        xc = x[c * BL:(c + 1) * BL]  # [32, T, 64]
        xfm = np.ascontiguousarray(xc.transpose(2, 1, 0)).reshape(64, T * BL)
        m = dict(shared)
        m["x_fm"] = xfm.astype(np.float16)
        in_maps.append(m)

    res = run_bass_kernel_spmd(nc, in_maps, list(range(N_CORES)))

    out = np.zeros((B, 256), np.float32)
    for c in range(N_CORES):
        o = res.results[c]["out"]  # [128, 64]
        out[c * BL:(c + 1) * BL, 0:128] = o[:, 0:32].T
        out[c * BL:(c + 1) * BL, 128:256] = o[:, 32:64].T
    return out


# revision 14
# speedup vs baseline: 2.8750x; 2.8750x over previous
"""CfC (closed-form continuous-time) 3-layer NCP encoder on 8 Trainium2 cores.

Strategy: data-parallel over batch (256 -> 32 per core), weights replicated.
Per core the T=1024 recurrence runs fully on-chip in FEATURE-MAJOR layout:
  - state s = [h0|h1|h2] (256 units) lives as [128 part, 64] fp16 tile:
    cols 0:32 = units 0:128 ("A"), cols 32:64 = units 128:256 ("B")
  - gate matmuls keep WEIGHTS stationary (M=128 cols -> FWL), state moving:
    PSUM tile [128, 192] = blocks [ff1A|ff1B|ff2A|ff2B|tA|tB]
  - x_t + bias contributions are separate K=65 matmuls (start=True) issued
    one iteration early so they hide under ACT/DVE of the previous step
  - sigmoid folded into tanh (t-weights pre-scaled 0.5): ONE activation
    instruction per step covers all 768 pre-activations
  - blend h = ff1 + (th+1)/2*(ff2-ff1) via 3 DVE ops (2 fused
    scalar_tensor_tensor), writing the next state tile directly
  - no transposes, no state copies; cross-engine chain PE->ACT->DVE->PE
"""
import sys, os

sys.path.insert(0, "/opt/trn_rl_repo")
os.environ.setdefault("JAX_PLATFORMS", "")
os.environ.setdefault("MYCRO_LOCAL_CACHE", "1")

import numpy as np

N_CORES = 8
B, T_FULL, D_IN = 256, 1024, 64
BL = B // N_CORES  # 32
H0, H1, H2 = 135, 89, 32
NCHUNK_STEPS = 16  # x steps per DMA chunk

_NC_CACHE = {}


def _build(T):
    REP = int(os.environ.get("REPEAT", "1"))
    DBG = os.environ.get("DBG", "") == "1"
    import concourse.tile as tile
    from concourse import bacc, mybir

    F32 = mybir.dt.float32
    F16 = mybir.dt.float16
    AF = mybir.ActivationFunctionType
    OP = mybir.AluOpType

    nc = bacc.Bacc("TRN2", target_bir_lowering=False, debug=False,
                   enable_asserts=True, num_devices=N_CORES)

    NCH = T // NCHUNK_STEPS
    assert T % NCHUNK_STEPS == 0

    d_x = nc.dram_tensor("x_fm", [64, T * BL], F16, kind="ExternalInput").ap()
    d_wx = nc.dram_tensor("wx", [65, 768], F16, kind="ExternalInput").ap()
    d_wa = nc.dram_tensor("wa", [128, 768], F16, kind="ExternalInput").ap()
    d_wb = nc.dram_tensor("wb", [128, 768], F16, kind="ExternalInput").ap()
    d_wfca = nc.dram_tensor("wfca", [128, 256], F16, kind="ExternalInput").ap()
    d_wfcb = nc.dram_tensor("wfcb", [128, 768], F16, kind="ExternalInput").ap()
    d_fcb = nc.dram_tensor("fcb", [128, 2], F32, kind="ExternalInput").ap()
    d_out = nc.dram_tensor("out", [128, 64], F32, kind="ExternalOutput").ap()
    if DBG:
        d_dxr = nc.dram_tensor("dbg_xr", [65, 512], F32, kind="ExternalOutput").ap()
        d_dg = nc.dram_tensor("dbg_g", [128, 192], F32, kind="ExternalOutput").ap()
        d_dst = nc.dram_tensor("dbg_st", [128, 64], F32, kind="ExternalOutput").ap()
        d_dpg = nc.dram_tensor("dbg_pg", [128, 192], F32, kind="ExternalOutput").ap()

    CW = 32 * NCHUNK_STEPS  # x-chunk width in cols

    with tile.TileContext(nc, trace_sim=False) as tc:
        with tc.tile_pool(name="persist", bufs=1) as pp, \
             tc.tile_pool(name="psum", bufs=1, space="PSUM") as psp:
            sWX = pp.tile([65, 768], F16)
            sWA = pp.tile([128, 768], F16)
            sWB = pp.tile([128, 768], F16)
            sWfca = pp.tile([128, 256], F16)
            sWfcb = pp.tile([128, 768], F16)
            sFcb = pp.tile([128, 2], F32)
            sXR = [pp.tile([65, CW], F16, name=f"XR{i}") for i in range(4)]
            sSt = [pp.tile([128, 64], F16, name=f"St{i}") for i in range(2)]
            sG = [pp.tile([128, 192], F16, name=f"G{i}") for i in range(2)]
            sD = [pp.tile([128, 64], F16, name=f"D{i}") for i in range(2)]
            sU = [pp.tile([128, 64], F16, name=f"U{i}") for i in range(2)]
            sFin = pp.tile([128, 64], F16)
            sOut = pp.tile([128, 64], F32)

            pG = [psp.tile([128, 192], F32, name=f"PG{i}") for i in range(2)]
            pFC = psp.tile([128, 64], F32)

            # --- load weights ---
            for dst, src in [(sWX, d_wx), (sWA, d_wa), (sWB, d_wb),
                             (sWfca, d_wfca), (sWfcb, d_wfcb), (sFcb, d_fcb)]:
                nc.sync.dma_start(dst[:], src)

            # --- init ---
            for s in sSt:
                nc.vector.memset(s[:], 0.0)
            for xr in sXR:
                nc.vector.memset(xr[64:65, :], 1.0)
            for c in range(min(2, NCH)):
                nc.sync.dma_start(sXR[c][0:64, :], d_x[:, c * CW:(c + 1) * CW])

            mm = nc.tensor.matmul

            import contextlib
            rep_ctx = tc.For_i(0, REP, 1) if REP > 1 else contextlib.nullcontext()
            with rep_ctx:
              for i in range(T + 2):
                p = i % 2
                w = (i + 1) % 2

                # prefetch x chunk two ahead
                if i % NCHUNK_STEPS == 0 and i // NCHUNK_STEPS + 2 < NCH:
                    c = i // NCHUNK_STEPS + 2
                    nc.sync.dma_start(sXR[c % 4][0:64, :],
                                      d_x[:, c * CW:(c + 1) * CW])

                # --- gate matmuls: weights stationary, state moving.
                # Per PSUM region the group is [X(start), A, B(stop)] kept
                # consecutive: start=True clears has_written for the WHOLE
                # bank, so interleaving groups loses accumulation state.
                # B-half regions (odd j) only have x-weights on their first
                # 7 units (h0 tail; biases are zero), so M=7 there — the
                # A-matmul's overwrite-where-unwritten covers units 7:128.
                xi = min(i, T - 1)
                slot = (xi // NCHUNK_STEPS) % 4
                c0 = (xi % NCHUNK_STEPS) * 32
                xs = sXR[slot][0:65, c0:c0 + 32]
                for j in range(6):
                    o = pG[p][:, j * 32:(j + 1) * 32]
                    mm(o, sWX[:, j * 128:(j + 1) * 128], xs,
                       start=True, stop=False)
                    mm(o, sWA[:, j * 128:(j + 1) * 128], sSt[p][:, 0:32],
                       start=False, stop=False)
                    mm(o, sWB[:, j * 128:(j + 1) * 128], sSt[p][:, 32:64],
                       start=False, stop=True)

                # --- one activation for all gates (sigmoid via tanh with
                # t-weights pre-scaled 0.5; avoids per-step act-table swaps) ---
                nc.scalar.activation(sG[p][:], pG[p][:], AF.Tanh)

                # --- blend: h = ff1 + (th+1)/2 * (ff2-ff1) ---
                nc.vector.tensor_sub(sD[p][:], sG[p][:, 64:128], sG[p][:, 0:64])
                nc.vector.scalar_tensor_tensor(
                    sU[p][:], sG[p][:, 128:192], 1.0, sD[p][:],
                    op0=OP.add, op1=OP.mult)
                nc.vector.scalar_tensor_tensor(
                    sSt[w][:], sU[p][:], 0.5, sG[p][:, 0:64],
                    op0=OP.mult, op1=OP.add)

                # snapshot state written by iteration T-1 (holds h0@{T-1})
                # before iteration T+1 overwrites that parity tile
                if i == T - 1:
                    nc.vector.tensor_copy(sFin[:], sSt[w][:])

                if DBG and i == 0:
                    sDbgPg = pp.tile([128, 192], F32, name="dbgpg")
                    sDbgG = pp.tile([128, 192], F32, name="dbgg")
                    sDbgSt = pp.tile([128, 64], F32, name="dbgst")
                    sDbgXr = pp.tile([65, 512], F32, name="dbgxr")
                    nc.scalar.activation(sDbgPg[:], pG[p][:], AF.Copy)
                    nc.sync.dma_start(d_dpg, sDbgPg[:])
                    nc.vector.tensor_copy(sDbgG[:], sG[p][:])
                    nc.sync.dma_start(d_dg, sDbgG[:])
                    nc.vector.tensor_copy(sDbgSt[:], sSt[w][:])
                    nc.sync.dma_start(d_dst, sDbgSt[:])
                    nc.vector.tensor_copy(sDbgXr[:], sXR[0][:, 0:512])
                    nc.sync.dma_start(d_dxr, sDbgXr[:])

            # --- final FC: out[u, b] = fc_W @ [h0|h1|h2] + fc_b ---
            # h0@{T-1} in sFin; h1@{T-1} in sSt[(T+1)%2] B rows 7:96;
            # h2@{T-1} in sSt[T%2] B rows 96:128.
            st_h1 = sSt[(T + 1) % 2]
            st_h2 = sSt[T % 2]
            for v in range(2):
                o = pFC[:, v * 32:(v + 1) * 32]
                mm(o, sWfca[:, v * 128:(v + 1) * 128], sFin[:, 0:32],
                   start=True, stop=False)
                mm(o, sWfcb[:, (0 + v) * 128:(1 + v) * 128], sFin[:, 32:64],
                   start=False, stop=False)
                mm(o, sWfcb[:, (2 + v) * 128:(3 + v) * 128], st_h1[:, 32:64],
                   start=False, stop=False)
                mm(o, sWfcb[:, (4 + v) * 128:(5 + v) * 128], st_h2[:, 32:64],
                   start=False, stop=True)
                nc.vector.tensor_scalar(sOut[:, v * 32:(v + 1) * 32], o,
                                        sFcb[:, v:v + 1], None, op0=OP.add)
            nc.sync.dma_start(d_out, sOut[:])

    nc.compile()
    return nc


def _gate_mats(inputs):
    """Dense per-gate maps M[g]: [256 state-units, 321 features]
    feature space = [x(0:64) | state(64:320) | bias(320)]."""
    f = np.float32
    M = {g: np.zeros((256, 321), f) for g in ("ff1", "ff2", "t")}
    layers = [
        (0, 64, 135, 0, 64),     # L0: input x @0, recurrent h0 @64
        (135, 135, 89, 64, 199),  # L1: input h0 @64, recurrent h1 @199
        (224, 89, 32, 199, 288),  # L2: input h1 @199, recurrent h2 @288
    ]
    for l, (r0, nin, hid, in_f0, rec_f0) in enumerate(layers):
        mask = np.asarray(inputs[f"mask{l}"], f)
        W1 = np.asarray(inputs[f"W1_{l}"], f) * mask
        W2 = np.asarray(inputs[f"W2_{l}"], f) * mask
        Wt = (np.asarray(inputs[f"Wa_{l}"], f) + np.asarray(inputs[f"Wb_{l}"], f)) * 0.5
        b1 = np.asarray(inputs[f"b1_{l}"], f)
        b2 = np.asarray(inputs[f"b2_{l}"], f)
        bt = (np.asarray(inputs[f"ba_{l}"], f) + np.asarray(inputs[f"bb_{l}"], f)) * 0.5
        for g, W, bb in (("ff1", W1, b1), ("ff2", W2, b2), ("t", Wt, bt)):
            M[g][r0:r0 + hid, in_f0:in_f0 + nin] = W[:, 0:nin]
            M[g][r0:r0 + hid, rec_f0:rec_f0 + hid] = W[:, nin:nin + hid]
            M[g][r0:r0 + hid, 320] = bb
    return M


def _prep_weights(inputs):
    f16 = np.float16
    f = np.float32
    M = _gate_mats(inputs)
    wx = np.zeros((65, 768), f)
    wa = np.zeros((128, 768), f)
    wb = np.zeros((128, 768), f)
    for gi, g in enumerate(("ff1", "ff2", "t")):
        for v in range(2):
            j = gi * 2 + v
            units = slice(v * 128, (v + 1) * 128)
            wx[0:64, j * 128:(j + 1) * 128] = M[g][units, 0:64].T
            wx[64, j * 128:(j + 1) * 128] = M[g][units, 320]
            wa[:, j * 128:(j + 1) * 128] = M[g][units, 64:192].T
            wb[:, j * 128:(j + 1) * 128] = M[g][units, 192:320].T

    fcW = np.asarray(inputs["fc_W"], f)  # [256, 256]
    fcb = np.asarray(inputs["fc_b"], f)
    wfca = np.zeros((128, 256), f)
    wfcb = np.zeros((128, 768), f)
    fcbt = np.zeros((128, 2), f)
    for v in range(2):
        rows = slice(v * 128, (v + 1) * 128)
        wfca[:, v * 128:(v + 1) * 128] = fcW[rows, 0:128].T
        full = fcW[rows, 128:256].T  # [128 K(state 128:256), 128 out]
        h0p = np.zeros((128, 128), f); h0p[0:7] = full[0:7]
        h1p = np.zeros((128, 128), f); h1p[7:96] = full[7:96]
        h2p = np.zeros((128, 128), f); h2p[96:128] = full[96:128]
        wfcb[:, (0 + v) * 128:(1 + v) * 128] = h0p
        wfcb[:, (2 + v) * 128:(3 + v) * 128] = h1p
        wfcb[:, (4 + v) * 128:(5 + v) * 128] = h2p
        fcbt[:, v] = fcb[rows]
    return {"wx": wx.astype(f16), "wa": wa.astype(f16), "wb": wb.astype(f16),
            "wfca": wfca.astype(f16), "wfcb": wfcb.astype(f16), "fcb": fcbt}


def kernel(**inputs):
    from concourse.bass_utils import run_bass_kernel_spmd

    T = inputs["x"].shape[1]
    key = (T, os.environ.get("REPEAT", "1"))
    if key not in _NC_CACHE:
        _NC_CACHE[key] = _build(T)
    nc = _NC_CACHE[key]

    shared = _prep_weights(inputs)
    x = np.asarray(inputs["x"], dtype=np.float32)
    in_maps = []
    for c in range(N_CORES):
        xc = x[c * BL:(c + 1) * BL]  # [32, T, 64]
        xfm = np.ascontiguousarray(xc.transpose(2, 1, 0)).reshape(64, T * BL)
        m = dict(shared)
        m["x_fm"] = xfm.astype(np.float16)
        in_maps.append(m)

    res = run_bass_kernel_spmd(nc, in_maps, list(range(N_CORES)))

    out = np.zeros((B, 256), np.float32)
    for c in range(N_CORES):
        o = res.results[c]["out"]  # [128, 64]
        out[c * BL:(c + 1) * BL, 0:128] = o[:, 0:32].T
        out[c * BL:(c + 1) * BL, 128:256] = o[:, 32:64].T
    return out
